# revision 60
# baseline (speedup 1.0000x reference)
"""Trainium2 Bass kernel for nn_AttentionNet (additive attention + masked softmax).

Math (per batch b):
    D[h, u] = (Wu @ W2)^T user + (bu@W2 + bs@W1)   [H, U]
    E[h, s] = (Ws[:6] @ W1)^T serv                 [H, S]
    u_i[u, s] = sum_h vt[h] * tanh(E[h, s] + D[h, u])
    probs[u, :] = softmax(10 * where(mask, u_i, log(1e-45)))

tanh(E+D) is factored with the tanh addition formula and a degree-3
Chebyshev expansion of 1/(1+p) (fit interval = data-driven bound on
|tanh(E-c)*tanh(D+c)| with per-channel shifts c_h):

    tanh(E+D) = (x+y)/(1+xy) ~= (x+y) sum_m c_m (x y)^m,
    x = tanh(E-c), y = tanh(D+c)

which separates into 5 PE matmul groups contracted over h, with the
output laid out [s, u] (S on PSUM partitions, U on the free dim):

    psum[s, u] = sum_j G_j[h, s]^T w_j[h, u]
      G_0 = r1           w_0 = (c0/c1) ones      (r_k = c1 vt x^k)
      G_1 = r2 - c1 vt   w_1 = y                 (c0 = -c1: symmetric fit)
      G_2 = k2 r1 - r3   w_2 = s2 y^2            (ACT Square, k2 = -c1/c2)
      G_3 = r4 - r2      w_3 = s2 y^3            (-c3/c2 = 1: symmetric fit)
      G_4 = q4 r3        w_4 = s2^2 y^4          (q4 = (c3/c1)/s2^2)

The device ships eb = exp(10 u_i - 3) (fp16, softmax-shift invariant);
the host applies the boolean mask and the row normalization (softmax
denominator) on the gathered fp32 output.

This version is RAW BASS (no TileContext): engine programs are emitted
explicitly with a handful of monotonic semaphores. This removes the
Tile framework's per-instruction semaphore traffic and its fixed
~7.5us full-semaphore-space teardown epilogue; our teardown is two
sem_clear ranges on the sync engine.
"""

import numpy as np
from contextlib import ExitStack

import concourse.bass as bass
import concourse.bacc as bacc
import concourse.mybir as mybir
from concourse.bass_utils import run_bass_kernel_spmd

F32 = mybir.dt.float32
F16 = mybir.dt.float16
AF = mybir.ActivationFunctionType
OP = mybir.AluOpType

N_CORES = 8
B, U, S, H = 16, 500, 256, 128
BC = B // N_CORES       # batches per core
UP = 512                # padded per-(b,chunk) stride in psum/eb
UW = 2 * UP - 12        # 1012: valid width of the fused per-core w tiles
EXP_BIAS = -3.0         # exp(10*u + EXP_BIAS): keeps eb in fp16 range

_CACHE = {}


def _coeffs(a):
    """Degree-3 Chebyshev interpolant of 1/(1+p) on [-a, a]."""
    k = np.arange(4)
    pk = a * np.cos((2 * k + 1) * np.pi / 8)
    return [float(v) for v in np.polyfit(pk, 1.0 / (1.0 + pk), 3)[::-1]]


def _build_nc(cc):
    c0, c1, c2, c3 = cc
    s2 = -c2 / c1
    nc = bacc.Bacc("TRN2", target_bir_lowering=False, debug=False)
    big16 = nc.dram_tensor(
        "big16", [6, 2 * H + BC * U + BC * S], F16, kind="ExternalInput")
    bv = nc.dram_tensor("bv", [H, 5], F32, kind="ExternalInput")
    out = nc.dram_tensor("eb", [H, BC * 2 * UP], F16, kind="ExternalOutput")

    with ExitStack() as ctx:
        def sb(nm, shape, dt=F16):
            return ctx.enter_context(nc.sbuf_tensor(nm, shape, dt))

        def ps(nm, shape):
            return ctx.enter_context(nc.psum_tensor(nm, shape, F32))

        big_sb = sb("big_sb", [6, 2 * H + BC * U + BC * S])
        bv_sb = sb("bv_sb", [H, 5], F32)
        wt = sb("wt", [H, UP])
        t0_sb = sb("t0_sb", [H, 4], F32)
        td = sb("td", [H, UW])
        w2 = sb("w2", [H, UW])
        w3 = sb("w3", [H, UW])
        w4 = sb("w4", [H, UW])
        te = sb("te", [H, BC * S])
        r1 = sb("r1", [H, BC * S])
        r2 = sb("r2", [H, BC * S])
        r3 = sb("r3", [H, BC * S])
        r4 = sb("r4", [H, BC * S])
        G1 = sb("G1", [H, BC * S])
        tG2 = sb("tG2", [H, BC * S])
        G2 = sb("G2", [H, BC * S])
        G3 = sb("G3", [H, BC * S])
        eb = sb("ebt", [H, BC * 2 * UP])
        scr = sb("scr", [1, 8])
        e_ps = ps("e_ps", [H, BC * S])
        d_ps = ps("d_ps", [H, 2 * UP])
        sps = ps("sps", [H, BC * 2 * UP])
        t0_ps = ps("t0_ps", [H, 4])

        sBIG = ctx.enter_context(nc.semaphore())
        sBV = ctx.enter_context(nc.semaphore())
        sBV2 = ctx.enter_context(nc.semaphore())
        sPE = ctx.enter_context(nc.semaphore())
        sACT = ctx.enter_context(nc.semaphore())
        sDVE = ctx.enter_context(nc.semaphore())
        sOUT = ctx.enter_context(nc.semaphore())
        sems = [sBIG, sBV, sBV2, sPE, sACT, sDVE, sOUT]
        nums = sorted(s.num for s in sems)
        assert nums == list(range(nums[0], nums[0] + len(sems))), nums

        bt_ap = bv_sb[:, 0:1]     # btot + c_h  (tanh-D bias)
        ncp = bv_sb[:, 1:2]       # -c_h        (tanh-E bias)
        c1vt_ap = bv_sb[:, 2:3]   # c1 * vt

        block = ctx.enter_context(nc.Block())

        WS = 2 * H + BC * S   # 768: end of the [w96 | sv] slice (E operands)

        @block.sync
        def _(sync):
            sync.dma_start(big_sb[0:6, 0:WS],
                           big16[0:6, 0:WS]).then_inc(sBIG, 16)
            for g in (0, 2, 3):
                sync.wait_ge(sACT, 7 + g)
                sync.dma_start(out[:, g * UP:(g + 1) * UP],
                               eb[:, g * UP:(g + 1) * UP]).then_inc(sOUT, 16)
            sync.wait_ge(sOUT, 64)
            sync.sem_clear(range(nums[0], nums[0] + len(sems)))

        @block.gpsimd
        def _(gpsimd):
            gpsimd.dma_start(bv_sb[:], bv[:]).then_inc(sBV2, 16)
            gpsimd.dma_start(big_sb[0:3, WS:],
                             big16[0:3, WS:]).then_inc(sBV, 16)
            gpsimd.wait_ge(sACT, 8)
            gpsimd.dma_start(out[:, UP:2 * UP],
                             eb[:, UP:2 * UP]).then_inc(sOUT, 16)

        @block.vector
        def _(vector):
            vector.memset(wt[:], 1.0).then_inc(sDVE, 1)            # -> 1
            vector.wait_ge(sACT, 1)                                # te
            vector.wait_ge(sBV2, 16)                               # bv tile
            vector.tensor_scalar_mul(r1[:], te[:], c1vt_ap).then_inc(sDVE, 1)
            vector.tensor_mul(r2[:], r1[:], te[:]).then_inc(sDVE, 1)   # -> 3
            # c0*vt = -(c1*vt) (symmetric fit)
            vector.tensor_scalar_sub(G1[:], r2[:], c1vt_ap).then_inc(sDVE, 1)
            vector.tensor_mul(r3[:], r2[:], te[:]).then_inc(sDVE, 1)   # -> 5
            vector.tensor_scalar_mul(tG2[:], r1[:], -c1 / c2)
            vector.tensor_sub(G2[:], tG2[:], r3[:]).then_inc(sDVE, 1)  # -> 6
            vector.tensor_mul(r4[:], r3[:], te[:]).then_inc(sDVE, 1)   # -> 7
            vector.tensor_sub(G3[:], r4[:], r2[:]).then_inc(sDVE, 1)   # -> 8
            vector.wait_ge(sACT, 5)                                # w2 full
            vector.tensor_mul(w3[:], td[:], w2[:]).then_inc(sDVE, 1)   # -> 9
            vector.wait_ge(sPE, 4)                                 # t0 psum
            # exp bias col: 10 * (c0/c1) * sum_h r1[h,s]  +  EXP_BIAS
            vector.tensor_scalar(
                t0_sb[:], t0_ps[:], 10.0 * c0 / c1, EXP_BIAS,
                OP.mult, OP.add).then_inc(sDVE, 1)                 # -> 10

        @block.scalar
        def _(scalar):
            # dependency-free dummy: forces the ACT table load to happen
            # during the input-DMA wait instead of on the critical path
            scalar.activation(scr[:], big_sb[0:1, 0:8], AF.Tanh)
            scalar.wait_ge(sBV2, 16)                               # bv tile
            scalar.wait_ge(sPE, 1)                                 # E-psum
            scalar.activation(te[:], e_ps[:], AF.Tanh,
                              bias=ncp).then_inc(sACT, 1)
            for b in range(BC):                # tanh-D per batch -> 2, 3
                scalar.wait_ge(sPE, 2 + b)
                o = b * UP
                wd = U if b == BC - 1 else UP
                scalar.activation(td[:, o:o + wd], d_ps[:, o:o + wd],
                                  AF.Tanh, bias=bt_ap).then_inc(sACT, 1)
            for b in range(BC):                # Square per batch -> 4, 5
                o = b * UP
                wd = U if b == BC - 1 else UP
                scalar.activation(w2[:, o:o + wd], td[:, o:o + wd], AF.Square,
                                  scale=float(np.sqrt(s2))).then_inc(sACT, 1)
            # w4 = (q4*w2)^2 with q4^2 = (c3/c1)/s2^2, so that G4 == r3
            scalar.activation(w4[:], w2[:], AF.Square,
                              scale=float(np.sqrt(c3 / c1) / s2)
                              ).then_inc(sACT, 1)                  # -> 6
            scalar.wait_ge(sDVE, 10)                               # t0 bias
            for g in range(BC * 2):
                o = g * UP
                scalar.wait_ge(sPE, 5 + g)                         # series g
                scalar.activation(eb[:, o:o + U], sps[:, o:o + U], AF.Exp,
                                  scale=10.0,
                                  bias=t0_sb[:, g:g + 1]).then_inc(sACT, 1)

        @block.tensor
        def _(tensor):
            # warmup matmuls while the input DMA flies: operand VALUES are
            # irrelevant for HAM warmth, so read the (garbage) eb tile with
            # no dependency at all -- PE starts the moment its queue opens
            for _ in range(4):
                tensor.matmul(sps[:, 0:UP], eb[:, 0:H], eb[:, 0:UP])
            tensor.wait_ge(sBIG, 16)
            tensor.matmul(e_ps[:], big_sb[0:6, H:2 * H],
                          big_sb[0:6, 2 * H:WS]).then_inc(sPE, 1)
            tensor.wait_ge(sBV, 16)     # ut landed (own semaphore)
            for b in range(BC):
                tensor.matmul(
                    d_ps[:, b * UP:b * UP + U], big_sb[0:3, 0:H],
                    big_sb[0:3, WS + b * U:WS + (b + 1) * U]
                ).then_inc(sPE, 1)                                 # -> 2, 3
            # fillers: keep the PE HAM activity window fed while the series
            # waits on tanh/chain results (e_ps bank is dead after te)
            tensor.wait_ge(sACT, 1)
            for _ in range(2):
                tensor.matmul(e_ps[:, 0:UP], eb[:, 0:H], eb[:, 0:UP])
            Gs = [None, G1, G2, G3, r3]
            ws = [None, td, w2, w3, w4]
            jwait = [None, (sDVE, 4), (sDVE, 6)]
            jwait2 = [None, (sACT, 2, 3), (sACT, 4, 5)]
            for j in (1, 2):
                tensor.wait_ge(*jwait[j])
                for g in range(BC * 2):
                    b, c = divmod(g, 2)
                    if c == 0:
                        tensor.wait_ge(jwait2[j][0], jwait2[j][1 + b])
                    tensor.matmul(
                        sps[:, g * UP:g * UP + U],
                        Gs[j][:, b * S + c * H:b * S + (c + 1) * H],
                        ws[j][:, b * UP:b * UP + U],
                        start=(j == 1), stop=False)
                if j == 1:
                    # Term0 bias columns: t0[s, g] = sum_h r1[h, s-chunk]
                    for g in range(BC * 2):
                        b, c = divmod(g, 2)
                        i = tensor.matmul(
                            t0_ps[:, g:g + 1],
                            r1[:, b * S + c * H:b * S + (c + 1) * H],
                            wt[:, 0:1], start=(g == 0), stop=True)
                    i.then_inc(sPE, 1)                             # -> 4
            # tail: per-group j3+j4 pairs so each group's exp can start as
            # soon as its own accumulation closes
            tensor.wait_ge(sDVE, 9)                                # w3 (G3<=)
            for g in range(BC * 2):
                b, c = divmod(g, 2)
                sl = slice(b * S + c * H, b * S + (c + 1) * H)
                su = slice(b * UP, b * UP + U)
                tensor.matmul(sps[:, g * UP:g * UP + U], G3[:, sl],
                              w3[:, su], start=False, stop=False)
                if g == 0:
                    tensor.wait_ge(sACT, 6)                        # w4
                tensor.matmul(sps[:, g * UP:g * UP + U], r3[:, sl],
                              w4[:, su], start=False,
                              stop=True).then_inc(sPE, 1)          # -> 5..8
    nc.compile()
    return nc


def _get_nc(cc):
    key = tuple(round(v, 9) for v in cc)
    if _CACHE.get("key") != key:
        _CACHE["nc"] = _build_nc(cc)
        _CACHE["key"] = key
    return _CACHE["nc"]


def _prep(user, serv, Wu, bu, Ws, bs, W1, W2, vt):
    wu32 = Wu @ W2                     # [3, H]
    ws32 = Ws[:6] @ W1                 # [6, H]
    btot = bu @ W2 + bs @ W1           # [H]
    # data-driven per-h ranges of D (incl. bias) and E -> optimal shifts c_h
    Dall = user[..., :3].reshape(-1, 3) @ wu32 + btot
    Eall = serv.reshape(-1, 6) @ ws32
    Dmin, Dmax = Dall.min(0), Dall.max(0)
    Emin, Emax = Eall.min(0), Eall.max(0)
    dp = 0.03 * (Dmax - Dmin) + 1e-3
    ep = 0.03 * (Emax - Emin) + 1e-3
    Dmin -= dp; Dmax += dp; Emin -= ep; Emax += ep
    cs = np.linspace(-2.0, 2.0, 1601)[:, None]
    xm = np.maximum(np.abs(np.tanh(Emax[None] - cs)),
                    np.abs(np.tanh(Emin[None] - cs)))
    ym = np.maximum(np.abs(np.tanh(Dmax[None] + cs)),
                    np.abs(np.tanh(Dmin[None] + cs)))
    prod = xm * ym
    c_h = cs[prod.argmin(0), 0].astype(np.float32)
    a = float(prod.min(0).max())
    cc = _coeffs(a)
    c1 = cc[1]

    w96 = np.zeros((6, 2 * H), np.float16)
    w96[0:3, 0:H] = wu32.astype(np.float16)
    w96[0:6, H:2 * H] = ws32.astype(np.float16)
    vt32 = vt.astype(np.float32)
    bvh = np.stack([btot + c_h, -c_h,
                    np.float32(c1) * vt32, 0.0 * vt32,
                    np.full(H, EXP_BIAS, np.float32)],
                   axis=1).astype(np.float32)          # [H, 5]
    userT = user[:, :, :3].transpose(0, 2, 1).astype(np.float16)  # [B,3,U]
    servT = serv.transpose(0, 2, 1).astype(np.float16)            # [B,6,S]
    in_maps = []
    for cid in range(N_CORES):
        sl = slice(cid * BC, (cid + 1) * BC)
        utc = userT[sl].transpose(1, 0, 2).reshape(3, BC * U)
        svc = servT[sl].transpose(1, 0, 2).reshape(6, BC * S)
        big = np.zeros((6, 2 * H + BC * U + BC * S), np.float16)
        big[:, 0:2 * H] = w96
        big[:, 2 * H:2 * H + BC * S] = svc
        big[0:3, 2 * H + BC * S:] = utc
        in_maps.append({"big16": big, "bv": np.ascontiguousarray(bvh)})
    return in_maps, cc


def kernel(user_input_seq_with_stay, server_input_seq, masks,
           Wu, bu, Ws, bs, W1, W2, vt, _trace=False):
    user = np.asarray(user_input_seq_with_stay, np.float32)
    serv = np.asarray(server_input_seq, np.float32)
    mk = np.asarray(masks)
    Wu = np.asarray(Wu, np.float32)
    bu = np.asarray(bu, np.float32)
    Ws = np.asarray(Ws, np.float32)
    bs = np.asarray(bs, np.float32)
    W1 = np.asarray(W1, np.float32)
    W2 = np.asarray(W2, np.float32)
    vt = np.asarray(vt, np.float32)

    in_maps, cc = _prep(user, serv, Wu, bu, Ws, bs, W1, W2, vt)
    nc = _get_nc(cc)
    res = run_bass_kernel_spmd(nc, in_maps, list(range(N_CORES)), trace=_trace)
    _CACHE["last"] = res
    outs = []
    for cid in range(N_CORES):
        o = res.results[cid]["eb"].astype(np.float32)   # [H, 2048]
        ebr = o.reshape(H, BC, 2, UP)[:, :, :, :U]      # [s_p, b, c, u]
        outs.append(ebr.transpose(1, 3, 2, 0).reshape(BC, U, S))
    full = np.concatenate(outs, axis=0)                 # [B, U, S] = exp vals
    m = mk.astype(np.float32)
    wgt = full * m
    probs = wgt / wgt.sum(-1, keepdims=True)
    return np.ascontiguousarray(probs.astype(np.float32))


# revision 61
# speedup vs baseline: 1.0231x; 1.0231x over previous
"""Trainium2 Bass kernel for nn_AttentionNet (additive attention + masked softmax).

Math (per batch b):
    D[h, u] = (Wu @ W2)^T user + (bu@W2 + bs@W1)   [H, U]
    E[h, s] = (Ws[:6] @ W1)^T serv                 [H, S]
    u_i[u, s] = sum_h vt[h] * tanh(E[h, s] + D[h, u])
    probs[u, :] = softmax(10 * where(mask, u_i, log(1e-45)))

tanh(E+D) is factored with the tanh addition formula and a degree-3
Chebyshev expansion of 1/(1+p) (fit interval = data-driven bound on
|tanh(E-c)*tanh(D+c)| with per-channel shifts c_h):

    tanh(E+D) = (x+y)/(1+xy) ~= (x+y) sum_m c_m (x y)^m,
    x = tanh(E-c), y = tanh(D+c)

which separates into 5 PE matmul groups contracted over h, with the
output laid out [s, u] (S on PSUM partitions, U on the free dim):

    psum[s, u] = sum_j G_j[h, s]^T w_j[h, u]
      G_0 = r1           w_0 = (c0/c1) ones      (r_k = c1 vt x^k)
      G_1 = r2 - c1 vt   w_1 = y                 (c0 = -c1: symmetric fit)
      G_2 = k2 r1 - r3   w_2 = s2 y^2            (ACT Square, k2 = -c1/c2)
      G_3 = r4 - r2      w_3 = s2 y^3            (-c3/c2 = 1: symmetric fit)
      G_4 = q4 r3        w_4 = s2^2 y^4          (q4 = (c3/c1)/s2^2)

The device ships eb = exp(10 u_i - 3) (fp16, softmax-shift invariant);
the host applies the boolean mask and the row normalization (softmax
denominator) on the gathered fp32 output.

This version is RAW BASS (no TileContext): engine programs are emitted
explicitly with a handful of monotonic semaphores. This removes the
Tile framework's per-instruction semaphore traffic and its fixed
~7.5us full-semaphore-space teardown epilogue; our teardown is two
sem_clear ranges on the sync engine.
"""

import numpy as np
from contextlib import ExitStack

import concourse.bass as bass
import concourse.bacc as bacc
import concourse.mybir as mybir
from concourse.bass_utils import run_bass_kernel_spmd

F32 = mybir.dt.float32
F16 = mybir.dt.float16
AF = mybir.ActivationFunctionType
OP = mybir.AluOpType

N_CORES = 8
B, U, S, H = 16, 500, 256, 128
BC = B // N_CORES       # batches per core
UP = 512                # padded per-(b,chunk) stride in psum/eb
UW = 2 * UP - 12        # 1012: valid width of the fused per-core w tiles
EXP_BIAS = -3.0         # exp(10*u + EXP_BIAS): keeps eb in fp16 range

_CACHE = {}


def _coeffs(a):
    """Degree-3 Chebyshev interpolant of 1/(1+p) on [-a, a]."""
    k = np.arange(4)
    pk = a * np.cos((2 * k + 1) * np.pi / 8)
    return [float(v) for v in np.polyfit(pk, 1.0 / (1.0 + pk), 3)[::-1]]


def _build_nc(cc):
    c0, c1, c2, c3 = cc
    s2 = -c2 / c1
    nc = bacc.Bacc("TRN2", target_bir_lowering=False, debug=False)
    big16 = nc.dram_tensor(
        "big16", [6, 2 * H + BC * U + BC * S], F16, kind="ExternalInput")
    bv = nc.dram_tensor("bv", [H, 5], F32, kind="ExternalInput")
    out = nc.dram_tensor("eb", [H, BC * 2 * UP], F16, kind="ExternalOutput")

    with ExitStack() as ctx:
        def sb(nm, shape, dt=F16):
            return ctx.enter_context(nc.sbuf_tensor(nm, shape, dt))

        def ps(nm, shape):
            return ctx.enter_context(nc.psum_tensor(nm, shape, F32))

        big_sb = sb("big_sb", [6, 2 * H + BC * U + BC * S])
        bv_sb = sb("bv_sb", [H, 5], F32)
        wt = sb("wt", [H, UP])
        t0_sb = sb("t0_sb", [H, 4], F32)
        td = sb("td", [H, UW])
        w2 = sb("w2", [H, UW])
        w3 = sb("w3", [H, UW])
        w4 = sb("w4", [H, UW])
        te = sb("te", [H, BC * S])
        r1 = sb("r1", [H, BC * S])
        r2 = sb("r2", [H, BC * S])
        r3 = sb("r3", [H, BC * S])
        r4 = sb("r4", [H, BC * S])
        G1 = sb("G1", [H, BC * S])
        tG2 = sb("tG2", [H, BC * S])
        G2 = sb("G2", [H, BC * S])
        G3 = sb("G3", [H, BC * S])
        eb = sb("ebt", [H, BC * 2 * UP])
        scr = sb("scr", [1, 8])
        e_ps = ps("e_ps", [H, BC * S])
        d_ps = ps("d_ps", [H, 2 * UP])
        sps = ps("sps", [H, BC * 2 * UP])
        t0_ps = ps("t0_ps", [H, 4])

        sBIG = ctx.enter_context(nc.semaphore())
        sBV = ctx.enter_context(nc.semaphore())
        sBV2 = ctx.enter_context(nc.semaphore())
        sPE = ctx.enter_context(nc.semaphore())
        sACT = ctx.enter_context(nc.semaphore())
        sDVE = ctx.enter_context(nc.semaphore())
        sOUT = ctx.enter_context(nc.semaphore())
        sems = [sBIG, sBV, sBV2, sPE, sACT, sDVE, sOUT]
        nums = sorted(s.num for s in sems)
        assert nums == list(range(nums[0], nums[0] + len(sems))), nums

        bt_ap = bv_sb[:, 0:1]     # btot + c_h  (tanh-D bias)
        ncp = bv_sb[:, 1:2]       # -c_h        (tanh-E bias)
        c1vt_ap = bv_sb[:, 2:3]   # c1 * vt

        block = ctx.enter_context(nc.Block())

        WS = 2 * H + BC * S   # 768: end of the [w96 | sv] slice (E operands)

        @block.sync
        def _(sync):
            sync.dma_start(big_sb[0:6, 0:WS],
                           big16[0:6, 0:WS]).then_inc(sBIG, 16)
            for g in (0, 2, 3):
                sync.wait_ge(sACT, 7 + g)
                sync.dma_start(out[:, g * UP:(g + 1) * UP],
                               eb[:, g * UP:(g + 1) * UP]).then_inc(sOUT, 16)
            sync.wait_ge(sOUT, 64)
            sync.sem_clear(range(nums[0], nums[0] + len(sems)))

        @block.gpsimd
        def _(gpsimd):
            gpsimd.dma_start(bv_sb[:], bv[:]).then_inc(sBV2, 16)
            gpsimd.dma_start(big_sb[0:3, WS:],
                             big16[0:3, WS:]).then_inc(sBV, 16)
            gpsimd.wait_ge(sACT, 8)
            gpsimd.dma_start(out[:, UP:2 * UP],
                             eb[:, UP:2 * UP]).then_inc(sOUT, 16)

        @block.vector
        def _(vector):
            vector.memset(wt[:], 1.0).then_inc(sDVE, 1)            # -> 1
            vector.wait_ge(sACT, 1)                                # te
            vector.wait_ge(sBV2, 16)                               # bv tile
            vector.tensor_scalar_mul(r1[:], te[:], c1vt_ap).then_inc(sDVE, 1)
            vector.tensor_mul(r2[:], r1[:], te[:]).then_inc(sDVE, 1)   # -> 3
            # c0*vt = -(c1*vt) (symmetric fit)
            vector.tensor_scalar_sub(G1[:], r2[:], c1vt_ap).then_inc(sDVE, 1)
            vector.tensor_mul(r3[:], r2[:], te[:]).then_inc(sDVE, 1)   # -> 5
            vector.tensor_scalar_mul(tG2[:], r1[:], -c1 / c2)
            vector.tensor_sub(G2[:], tG2[:], r3[:]).then_inc(sDVE, 1)  # -> 6
            vector.tensor_mul(r4[:], r3[:], te[:]).then_inc(sDVE, 1)   # -> 7
            vector.tensor_sub(G3[:], r4[:], r2[:]).then_inc(sDVE, 1)   # -> 8
            vector.wait_ge(sACT, 5)                                # w2 full
            vector.tensor_mul(w3[:], td[:], w2[:]).then_inc(sDVE, 1)   # -> 9
            vector.wait_ge(sPE, 4)                                 # t0 psum
            # exp bias col: 10 * (c0/c1) * sum_h r1[h,s]  +  EXP_BIAS
            vector.tensor_scalar(
                t0_sb[:], t0_ps[:], 10.0 * c0 / c1, EXP_BIAS,
                OP.mult, OP.add).then_inc(sDVE, 1)                 # -> 10

        @block.scalar
        def _(scalar):
            # dependency-free dummy: forces the ACT table load to happen
            # during the input-DMA wait instead of on the critical path
            scalar.activation(scr[:], big_sb[0:1, 0:8], AF.Tanh)
            scalar.wait_ge(sBV2, 16)                               # bv tile
            scalar.wait_ge(sPE, 1)                                 # E-psum
            scalar.activation(te[:], e_ps[:], AF.Tanh,
                              bias=ncp).then_inc(sACT, 1)
            for b in range(BC):                # tanh-D per batch -> 2, 3
                scalar.wait_ge(sPE, 2 + b)
                o = b * UP
                wd = U if b == BC - 1 else UP
                scalar.activation(td[:, o:o + wd], d_ps[:, o:o + wd],
                                  AF.Tanh, bias=bt_ap).then_inc(sACT, 1)
            for b in range(BC):                # Square per batch -> 4, 5
                o = b * UP
                wd = U if b == BC - 1 else UP
                scalar.activation(w2[:, o:o + wd], td[:, o:o + wd], AF.Square,
                                  scale=float(np.sqrt(s2))).then_inc(sACT, 1)
            # w4 = (q4*w2)^2 with q4^2 = (c3/c1)/s2^2, so that G4 == r3
            scalar.activation(w4[:], w2[:], AF.Square,
                              scale=float(np.sqrt(c3 / c1) / s2)
                              ).then_inc(sACT, 1)                  # -> 6
            scalar.wait_ge(sDVE, 10)                               # t0 bias
            for g in range(BC * 2):
                o = g * UP
                scalar.wait_ge(sPE, 5 + g)                         # series g
                scalar.activation(eb[:, o:o + U], sps[:, o:o + U], AF.Exp,
                                  scale=10.0,
                                  bias=t0_sb[:, g:g + 1]).then_inc(sACT, 1)

        @block.tensor
        def _(tensor):
            # warmup matmuls while the input DMA flies: operand VALUES are
            # irrelevant for HAM warmth, so read the (garbage) eb tile with
            # no dependency at all -- PE starts the moment its queue opens
            for _ in range(4):
                tensor.matmul(sps[:, 0:UP], eb[:, 0:H], eb[:, 0:UP])
            tensor.wait_ge(sBIG, 16)
            tensor.matmul(e_ps[:], big_sb[0:6, H:2 * H],
                          big_sb[0:6, 2 * H:WS]).then_inc(sPE, 1)
            tensor.wait_ge(sBV, 16)     # ut landed (own semaphore)
            for b in range(BC):
                tensor.matmul(
                    d_ps[:, b * UP:b * UP + U], big_sb[0:3, 0:H],
                    big_sb[0:3, WS + b * U:WS + (b + 1) * U]
                ).then_inc(sPE, 1)                                 # -> 2, 3
            # fillers: keep the PE HAM activity window fed while the series
            # waits on tanh/chain results (e_ps bank is dead after te)
            tensor.wait_ge(sACT, 1)
            for _ in range(2):
                tensor.matmul(e_ps[:, 0:UP], eb[:, 0:H], eb[:, 0:UP])
            Gs = [None, G1, G2, G3, r3]
            ws = [None, td, w2, w3, w4]
            jwait = [None, (sDVE, 4), (sDVE, 6)]
            jwait2 = [None, (sACT, 2, 3), (sACT, 4, 5)]
            for j in (1, 2):
                tensor.wait_ge(*jwait[j])
                for g in range(BC * 2):
                    b, c = divmod(g, 2)
                    if c == 0:
                        tensor.wait_ge(jwait2[j][0], jwait2[j][1 + b])
                    tensor.matmul(
                        sps[:, g * UP:g * UP + U],
                        Gs[j][:, b * S + c * H:b * S + (c + 1) * H],
                        ws[j][:, b * UP:b * UP + U],
                        start=(j == 1), stop=False)
                if j == 1:
                    # Term0 bias columns: t0[s, g] = sum_h r1[h, s-chunk]
                    for g in range(BC * 2):
                        b, c = divmod(g, 2)
                        i = tensor.matmul(
                            t0_ps[:, g:g + 1],
                            r1[:, b * S + c * H:b * S + (c + 1) * H],
                            wt[:, 0:1], start=(g == 0), stop=True)
                    i.then_inc(sPE, 1)                             # -> 4
            # tail: per-group j3+j4 pairs so each group's exp can start as
            # soon as its own accumulation closes (the extra filler here also
            # keeps the HAM activity window fed -- measured net-positive)
            tensor.matmul(e_ps[:, 0:UP], eb[:, 0:H], eb[:, 0:UP])
            tensor.wait_ge(sDVE, 9)                                # w3 (G3<=)
            for g in range(BC * 2):
                b, c = divmod(g, 2)
                sl = slice(b * S + c * H, b * S + (c + 1) * H)
                su = slice(b * UP, b * UP + U)
                tensor.matmul(sps[:, g * UP:g * UP + U], G3[:, sl],
                              w3[:, su], start=False, stop=False)
                if g == 0:
                    tensor.wait_ge(sACT, 6)                        # w4
                tensor.matmul(sps[:, g * UP:g * UP + U], r3[:, sl],
                              w4[:, su], start=False,
                              stop=True).then_inc(sPE, 1)          # -> 5..8
    nc.compile()
    return nc


def _get_nc(cc):
    key = tuple(round(v, 9) for v in cc)
    if _CACHE.get("key") != key:
        _CACHE["nc"] = _build_nc(cc)
        _CACHE["key"] = key
    return _CACHE["nc"]


def _prep(user, serv, Wu, bu, Ws, bs, W1, W2, vt):
    wu32 = Wu @ W2                     # [3, H]
    ws32 = Ws[:6] @ W1                 # [6, H]
    btot = bu @ W2 + bs @ W1           # [H]
    # data-driven per-h ranges of D (incl. bias) and E -> optimal shifts c_h
    Dall = user[..., :3].reshape(-1, 3) @ wu32 + btot
    Eall = serv.reshape(-1, 6) @ ws32
    Dmin, Dmax = Dall.min(0), Dall.max(0)
    Emin, Emax = Eall.min(0), Eall.max(0)
    dp = 0.03 * (Dmax - Dmin) + 1e-3
    ep = 0.03 * (Emax - Emin) + 1e-3
    Dmin -= dp; Dmax += dp; Emin -= ep; Emax += ep
    cs = np.linspace(-2.0, 2.0, 1601)[:, None]
    xm = np.maximum(np.abs(np.tanh(Emax[None] - cs)),
                    np.abs(np.tanh(Emin[None] - cs)))
    ym = np.maximum(np.abs(np.tanh(Dmax[None] + cs)),
                    np.abs(np.tanh(Dmin[None] + cs)))
    prod = xm * ym
    c_h = cs[prod.argmin(0), 0].astype(np.float32)
    a = float(prod.min(0).max())
    cc = _coeffs(a)
    c1 = cc[1]

    w96 = np.zeros((6, 2 * H), np.float16)
    w96[0:3, 0:H] = wu32.astype(np.float16)
    w96[0:6, H:2 * H] = ws32.astype(np.float16)
    vt32 = vt.astype(np.float32)
    bvh = np.stack([btot + c_h, -c_h,
                    np.float32(c1) * vt32, 0.0 * vt32,
                    np.full(H, EXP_BIAS, np.float32)],
                   axis=1).astype(np.float32)          # [H, 5]
    userT = user[:, :, :3].transpose(0, 2, 1).astype(np.float16)  # [B,3,U]
    servT = serv.transpose(0, 2, 1).astype(np.float16)            # [B,6,S]
    in_maps = []
    for cid in range(N_CORES):
        sl = slice(cid * BC, (cid + 1) * BC)
        utc = userT[sl].transpose(1, 0, 2).reshape(3, BC * U)
        svc = servT[sl].transpose(1, 0, 2).reshape(6, BC * S)
        big = np.zeros((6, 2 * H + BC * U + BC * S), np.float16)
        big[:, 0:2 * H] = w96
        big[:, 2 * H:2 * H + BC * S] = svc
        big[0:3, 2 * H + BC * S:] = utc
        in_maps.append({"big16": big, "bv": np.ascontiguousarray(bvh)})
    return in_maps, cc


def kernel(user_input_seq_with_stay, server_input_seq, masks,
           Wu, bu, Ws, bs, W1, W2, vt, _trace=False):
    user = np.asarray(user_input_seq_with_stay, np.float32)
    serv = np.asarray(server_input_seq, np.float32)
    mk = np.asarray(masks)
    Wu = np.asarray(Wu, np.float32)
    bu = np.asarray(bu, np.float32)
    Ws = np.asarray(Ws, np.float32)
    bs = np.asarray(bs, np.float32)
    W1 = np.asarray(W1, np.float32)
    W2 = np.asarray(W2, np.float32)
    vt = np.asarray(vt, np.float32)

    in_maps, cc = _prep(user, serv, Wu, bu, Ws, bs, W1, W2, vt)
    nc = _get_nc(cc)
    res = run_bass_kernel_spmd(nc, in_maps, list(range(N_CORES)), trace=_trace)
    _CACHE["last"] = res
    outs = []
    for cid in range(N_CORES):
        o = res.results[cid]["eb"].astype(np.float32)   # [H, 2048]
        ebr = o.reshape(H, BC, 2, UP)[:, :, :, :U]      # [s_p, b, c, u]
        outs.append(ebr.transpose(1, 3, 2, 0).reshape(BC, U, S))
    full = np.concatenate(outs, axis=0)                 # [B, U, S] = exp vals
    m = mk.astype(np.float32)
    wgt = full * m
    probs = wgt / wgt.sum(-1, keepdims=True)
    return np.ascontiguousarray(probs.astype(np.float32))
